# revision 16
# baseline (speedup 1.0000x reference)
"""DoubleAttention TRN2 Bass kernel.

Full inputs in, full outputs out. Data-parallel over batch: B=32 split as
4 batches per core across 8 NeuronCores; 1x1-conv weights replicated.

Per-batch math (C = Cout = dn = 512, N = H*W = 1024):
  A   = wA @ x + bA            [C, N]
  smB = softmax(wB @ x, n)     (bB drops: softmax shift-invariant)
  smV = softmax(wV @ x, n)     (bV drops)
  G   = A @ smB^T              [C, C]
  Z   = wR @ (G @ smV) + bR    [C, N]

Kernel-side formulation (everything float32r on the PE):
  AT[n,c]   = sum_c' x[c',n] wA^T[c',c]        (transposed conv; no transposes
  EBT[n,d]  = exp(sum_c' x[c',n] wB^T[c',d])    needed for the n-contraction)
  EV[d,n]   = exp(Vm[d,n]); sV[d] = sum_n EV[d,n]   (natural layout)
  sB[d]     = sum_n EBT[n,d]    via ones-matmul + rank-1 transpose matmuls
  GrawT[d,c]= sum_n EBT[n,d] AT[n,c] + sB[d]*bA[c]  (rank-1 K=1 matmul update)
  GT[d,c]   = GrawT[d,c] / (sB[d]*sV[d])        (per-partition scale on evac)
  Z0[c,n]   = sum_d GT[d,c] EV[d,n]
  out[o,n]  = sum_c wR^T[c,o] Z0[c,n] + bR[o]   (bias via ACT activation)
"""

import numpy as np

B, C, N = 32, 512, 1024  # batch, channels, spatial (32*32)
H = W = 32
NCORES = 8
BPC = B // NCORES   # batches per core
KT = C // 128       # 4 contraction tiles
NT = N // 128       # 8 n-partition tiles
NS = N // 512       # 2 n free-dim spans

_CACHE = {}


def _build_nc():
    import concourse.bacc as bacc
    import concourse.mybir as mybir
    import concourse.tile as tile

    F32 = mybir.dt.float32
    F32R = mybir.dt.float32r
    AF = mybir.ActivationFunctionType

    nc = bacc.Bacc("TRN2", target_bir_lowering=False, debug=False,
                   num_devices=NCORES)
    x_d = nc.dram_tensor("x", [BPC, C, N], F32R, kind="ExternalInput").ap()
    wat_d = nc.dram_tensor("wat", [C, C], F32R, kind="ExternalInput").ap()
    wbt_d = nc.dram_tensor("wbt", [C, C], F32R, kind="ExternalInput").ap()
    wvt_d = nc.dram_tensor("wvt", [C, C], F32R, kind="ExternalInput").ap()
    wrt_d = nc.dram_tensor("wrt", [C, C], F32R, kind="ExternalInput").ap()
    bab_d = nc.dram_tensor("bab", [128, C], F32, kind="ExternalInput").ap()
    br_d = nc.dram_tensor("br", [128, KT], F32, kind="ExternalInput").ap()
    ones_d = nc.dram_tensor("ones", [128, 128], F32R, kind="ExternalInput").ap()
    o_d = nc.dram_tensor("o", [BPC, C, N], F32, kind="ExternalOutput").ap()

    with tile.TileContext(nc) as tc:
        with tc.tile_pool(name="wp", bufs=1) as wp, \
             tc.tile_pool(name="xp", bufs=2) as xp, \
             tc.tile_pool(name="ip", bufs=1) as ip, \
             tc.tile_pool(name="op", bufs=2) as op_, \
             tc.tile_pool(name="sp", bufs=2) as sp, \
             tc.tile_pool(name="pp", bufs=7, space="PSUM") as pp, \
             tc.tile_pool(name="pq", bufs=1, space="PSUM") as pq:

            wat = wp.tile([128, KT, C], F32R, tag="wat")
            wbt = wp.tile([128, KT, C], F32R, tag="wbt")
            wvt = wp.tile([128, KT, C], F32R, tag="wvt")
            wrt = wp.tile([128, KT, C], F32R, tag="wrt")
            xs0 = xp.tile([128, KT, N], F32R, tag="xs")
            ones = wp.tile([128, 128], F32R, tag="ones")
            nc.sync.dma_start(ones[:], ones_d[:])
            # Warm the PE HAM clock gate during the DMA head: ~4us of dummy
            # matmuls so the real stream starts at 2.4 GHz.
            psw = pq.tile([128, 512], F32, tag="q")
            for _ in range(40):
                nc.tensor.matmul(psw[:, 0:128], ones[:], ones[:],
                                 start=True, stop=True)
            # DMA priority order for batch 0: the first PV group needs
            # x[:, :, 0:512] plus wvt. Medium chunks on alternating queues
            # maximize early aggregate bandwidth without flooding the SP
            # sequencer with triggers.
            for k in range(KT):
                nc.sync.dma_start(xs0[:, k, 0:512],
                                  x_d[0, k * 128:(k + 1) * 128, 0:512])
                nc.sync.dma_start(wvt[:, k, :],
                                  wvt_d[k * 128:(k + 1) * 128, :])
            nc.sync.dma_start(xs0[:, :, 512:1024],
                              x_d[0, :, 512:1024].rearrange(
                                  "(k p) n -> p k n", p=128))
            for k in range(KT):
                nc.sync.dma_start(wat[:, k, :], wat_d[k * 128:(k + 1) * 128, :])
                nc.sync.dma_start(wbt[:, k, :], wbt_d[k * 128:(k + 1) * 128, :])
            nc.sync.dma_start(wrt[:], wrt_d.rearrange("(k p) c -> p k c",
                                                      p=128))
            bab = wp.tile([128, C], F32, tag="bab")
            nc.sync.dma_start(bab[:], bab_d[:])
            br = wp.tile([128, KT], F32, tag="br")
            nc.sync.dma_start(br[:], br_d[:])

            for b in range(BPC):
                if b == 0:
                    xs = xs0
                else:
                    xs = xp.tile([128, KT, N], F32R, tag="xs")
                    for h in range(NS):
                        hsl = slice(h * 512, (h + 1) * 512)
                        nc.sync.dma_start(
                            xs[:, :, hsl],
                            x_d[b, :, hsl].rearrange("(k p) n -> p k n",
                                                     p=128))

                at = ip.tile([128, NT, C], F32R, tag="at")
                ebt = ip.tile([128, NT, C], F32R, tag="ebt")
                ev = ip.tile([128, KT, N], F32R, tag="ev")
                gt = ip.tile([128, KT, C], F32R, tag="gt")
                zs = ip.tile([128, KT, N], F32R, tag="zs")
                av = sp.tile([128, KT, NS], F32, tag="av")
                svc = sp.tile([128, KT], F32, tag="svc")
                sbc = sp.tile([128, KT], F32, tag="sbc")
                prod = sp.tile([128, KT], F32, tag="prod")
                rsc = sp.tile([128, KT], F32, tag="rsc")
                rsv = sp.tile([128, KT], F32, tag="rsv")
                sbr = sp.tile([1, C], F32R, tag="sbr")
                os_ = op_.tile([128, KT, N], F32, tag="os")

                # Phase V: EV[d,n] natural + per-row expsums (h outer so
                # the first groups only need the first half of x)
                for h in range(NS):
                    hsl = slice(h * 512, (h + 1) * 512)
                    for dt in range(KT):
                        dsl = slice(dt * 128, (dt + 1) * 128)
                        psv = pp.tile([128, 512], F32, tag="mm")
                        for k in range(KT):
                            nc.tensor.matmul(psv[:], wvt[:, k, dsl],
                                             xs[:, k, hsl],
                                             start=(k == 0), stop=(k == KT - 1))
                        nc.scalar.activation(ev[:, dt, hsl], psv[:], AF.Exp,
                                             accum_out=av[:, dt, h:h + 1])
                nc.vector.tensor_add(svc[:], av[:, :, 0], av[:, :, 1])
                nc.vector.reciprocal(rsv[:], svc[:])

                # Phase 1: AT[n,c] and EBT[n,d] per n-tile.
                # k-paired order: consecutive matmuls share the same stationary
                # xs chunk (one weight set serves psa and psb).
                for nt in range(NT):
                    nsl = slice(nt * 128, (nt + 1) * 128)
                    psa = pp.tile([128, C], F32, tag="mm")
                    psb = pp.tile([128, C], F32, tag="mm")
                    for k in range(KT):
                        nc.tensor.matmul(psa[:], xs[:, k, nsl], wat[:, k, :],
                                         start=(k == 0), stop=(k == KT - 1))
                        nc.tensor.matmul(psb[:], xs[:, k, nsl], wbt[:, k, :],
                                         start=(k == 0), stop=(k == KT - 1))
                    nc.vector.tensor_copy(at[:, nt, :], psa[:])
                    nc.scalar.activation(ebt[:, nt, :], psb[:], AF.Exp)

                # Phase 2: sB row via ones-matmul, then to column via K=1 matmuls
                pss = pq.tile([128, 512], F32, tag="q")
                for nt in range(NT):
                    nc.tensor.matmul(pss[:], ones[:], ebt[:, nt, :],
                                     start=(nt == 0), stop=(nt == NT - 1))
                nc.vector.tensor_copy(sbr[:], pss[0:1, :])

                # Phase G: GrawT[d,c]; evac folds the 1/(sB*sV) scale and the
                # +bA[c] bias (GT = GrawT*rscale + bA_bcast*rsV, rank-1-free).
                # The sB row->column transpose matmuls slot in after the first
                # group so their sbr/rsc dependency chain hides under PG.
                for dt in range(KT):
                    dsl = slice(dt * 128, (dt + 1) * 128)
                    psg = pp.tile([128, C], F32, tag="mm")
                    for nt in range(NT):
                        nc.tensor.matmul(psg[:], ebt[:, nt, dsl], at[:, nt, :],
                                         start=(nt == 0), stop=(nt == NT - 1))
                    if dt == 0:
                        psc = pq.tile([128, KT, 2], F32, tag="q")
                        for dtc in range(KT):
                            nc.tensor.matmul(
                                psc[:, dtc, :],
                                sbr[0:1, dtc * 128:(dtc + 1) * 128],
                                ones[0:1, 0:2], start=True, stop=True)
                        nc.vector.tensor_copy(sbc[:], psc[:, :, 0])
                        nc.vector.tensor_mul(prod[:], sbc[:], svc[:])
                        nc.vector.reciprocal(rsc[:], prod[:])
                    gta = sp.tile([128, C], F32, tag="gta")
                    nc.scalar.mul(gta[:], psg[:], rsc[:, dt:dt + 1])
                    tmpb = sp.tile([128, C], F32, tag="tmpb")
                    nc.vector.tensor_scalar_mul(tmpb[:], bab[:],
                                                rsv[:, dt:dt + 1])
                    nc.vector.tensor_add(gt[:, dt, :], gta[:], tmpb[:])

                # Phase Z: Z0[c,n]
                for ct in range(KT):
                    csl = slice(ct * 128, (ct + 1) * 128)
                    for h in range(NS):
                        hsl = slice(h * 512, (h + 1) * 512)
                        psz = pp.tile([128, 512], F32, tag="mm")
                        for dt in range(KT):
                            nc.tensor.matmul(psz[:], gt[:, dt, csl],
                                             ev[:, dt, hsl],
                                             start=(dt == 0), stop=(dt == KT - 1))
                        nc.vector.tensor_copy(zs[:, ct, hsl], psz[:])

                # Phase R: out[o,n] = wR @ Z + bR
                for ot in range(KT):
                    osl = slice(ot * 128, (ot + 1) * 128)
                    for h in range(NS):
                        hsl = slice(h * 512, (h + 1) * 512)
                        psr = pp.tile([128, 512], F32, tag="mm")
                        for k in range(KT):
                            nc.tensor.matmul(psr[:], wrt[:, k, osl],
                                             zs[:, k, hsl],
                                             start=(k == 0), stop=(k == KT - 1))
                        nc.scalar.activation(os_[:, ot, hsl], psr[:],
                                             AF.Identity, bias=br[:, ot:ot + 1])
                        nc.sync.dma_start(
                            o_d[b, ot * 128:(ot + 1) * 128, h * 512:(h + 1) * 512],
                            os_[:, ot, hsl])
    nc.compile()
    return nc


def _in_maps(x, wA, bA, wB, wV, wR, bR):
    xr = np.ascontiguousarray(x.reshape(B, C, N), dtype=np.float32)
    wat = np.ascontiguousarray(wA.T, dtype=np.float32)
    wbt = np.ascontiguousarray(wB.T, dtype=np.float32)
    wvt = np.ascontiguousarray(wV.T, dtype=np.float32)
    wrt = np.ascontiguousarray(wR.T, dtype=np.float32)
    bab = np.ascontiguousarray(
        np.broadcast_to(bA.reshape(1, C), (128, C)), dtype=np.float32)
    br = np.ascontiguousarray(bR.reshape(KT, 128).T, dtype=np.float32)
    ones = np.ones((128, 128), dtype=np.float32)
    maps = []
    for i in range(NCORES):
        maps.append({
            "x": np.ascontiguousarray(xr[i * BPC:(i + 1) * BPC]),
            "wat": wat, "wbt": wbt, "wvt": wvt, "wrt": wrt,
            "bab": bab, "br": br, "ones": ones,
        })
    return maps


def kernel(x, wA, bA, wB, bB, wV, bV, wR, bR):
    from concourse.bass_utils import run_bass_kernel_spmd
    if "nc" not in _CACHE:
        _CACHE["nc"] = _build_nc()
    nc = _CACHE["nc"]
    maps = _in_maps(x, wA, bA, wB, wV, wR, bR)
    res = run_bass_kernel_spmd(nc, maps, list(range(NCORES)))
    out = np.concatenate([res.results[i]["o"] for i in range(NCORES)], axis=0)
    return out.reshape(B, C, H, W).astype(np.float32)


# revision 17
# speedup vs baseline: 1.0059x; 1.0059x over previous
"""DoubleAttention TRN2 Bass kernel.

Full inputs in, full outputs out. Data-parallel over batch: B=32 split as
4 batches per core across 8 NeuronCores; 1x1-conv weights replicated.

Per-batch math (C = Cout = dn = 512, N = H*W = 1024):
  A   = wA @ x + bA            [C, N]
  smB = softmax(wB @ x, n)     (bB drops: softmax shift-invariant)
  smV = softmax(wV @ x, n)     (bV drops)
  G   = A @ smB^T              [C, C]
  Z   = wR @ (G @ smV) + bR    [C, N]

Kernel-side formulation (everything float32r on the PE):
  AT[n,c]   = sum_c' x[c',n] wA^T[c',c]        (transposed conv; no transposes
  EBT[n,d]  = exp(sum_c' x[c',n] wB^T[c',d])    needed for the n-contraction)
  EV[d,n]   = exp(Vm[d,n]); sV[d] = sum_n EV[d,n]   (natural layout)
  sB[d]     = sum_n EBT[n,d]    via ones-matmul + rank-1 transpose matmuls
  GrawT[d,c]= sum_n EBT[n,d] AT[n,c] + sB[d]*bA[c]  (rank-1 K=1 matmul update)
  GT[d,c]   = GrawT[d,c] / (sB[d]*sV[d])        (per-partition scale on evac)
  Z0[c,n]   = sum_d GT[d,c] EV[d,n]
  out[o,n]  = sum_c wR^T[c,o] Z0[c,n] + bR[o]   (bias via ACT activation)
"""

import numpy as np

B, C, N = 32, 512, 1024  # batch, channels, spatial (32*32)
H = W = 32
NCORES = 8
BPC = B // NCORES   # batches per core
KT = C // 128       # 4 contraction tiles
NT = N // 128       # 8 n-partition tiles
NS = N // 512       # 2 n free-dim spans

_CACHE = {}


def _build_nc():
    import concourse.bacc as bacc
    import concourse.mybir as mybir
    import concourse.tile as tile

    F32 = mybir.dt.float32
    F32R = mybir.dt.float32r
    AF = mybir.ActivationFunctionType

    nc = bacc.Bacc("TRN2", target_bir_lowering=False, debug=False,
                   num_devices=NCORES)
    x_d = nc.dram_tensor("x", [BPC, C, N], F32R, kind="ExternalInput").ap()
    wat_d = nc.dram_tensor("wat", [C, C], F32R, kind="ExternalInput").ap()
    wbt_d = nc.dram_tensor("wbt", [C, C], F32R, kind="ExternalInput").ap()
    wvt_d = nc.dram_tensor("wvt", [C, C], F32R, kind="ExternalInput").ap()
    wrt_d = nc.dram_tensor("wrt", [C, C], F32R, kind="ExternalInput").ap()
    bab_d = nc.dram_tensor("bab", [128, C], F32, kind="ExternalInput").ap()
    br_d = nc.dram_tensor("br", [128, KT], F32, kind="ExternalInput").ap()
    ones_d = nc.dram_tensor("ones", [128, 128], F32R, kind="ExternalInput").ap()
    o_d = nc.dram_tensor("o", [BPC, C, N], F32, kind="ExternalOutput").ap()

    with tile.TileContext(nc) as tc:
        with tc.tile_pool(name="wp", bufs=1) as wp, \
             tc.tile_pool(name="xp", bufs=2) as xp, \
             tc.tile_pool(name="ip", bufs=1) as ip, \
             tc.tile_pool(name="ip2", bufs=2) as ip2, \
             tc.tile_pool(name="op", bufs=2) as op_, \
             tc.tile_pool(name="sp", bufs=2) as sp, \
             tc.tile_pool(name="pp", bufs=7, space="PSUM") as pp, \
             tc.tile_pool(name="pq", bufs=1, space="PSUM") as pq:

            wat = wp.tile([128, KT, C], F32R, tag="wat")
            wbt = wp.tile([128, KT, C], F32R, tag="wbt")
            wvt = wp.tile([128, KT, C], F32R, tag="wvt")
            wrt = wp.tile([128, KT, C], F32R, tag="wrt")
            xs0 = xp.tile([128, KT, N], F32R, tag="xs")
            ones = wp.tile([128, 128], F32R, tag="ones")
            nc.sync.dma_start(ones[:], ones_d[:])
            # DMA priority order for batch 0: the first PV group needs
            # x[:, :, 0:512] plus wvt. Medium chunks on alternating queues
            # maximize early aggregate bandwidth without flooding the SP
            # sequencer with triggers.
            for k in range(KT):
                nc.sync.dma_start(xs0[:, k, 0:512],
                                  x_d[0, k * 128:(k + 1) * 128, 0:512])
                nc.sync.dma_start(wvt[:, k, :],
                                  wvt_d[k * 128:(k + 1) * 128, :])
            nc.sync.dma_start(xs0[:, :, 512:1024],
                              x_d[0, :, 512:1024].rearrange(
                                  "(k p) n -> p k n", p=128))
            for k in range(KT):
                nc.sync.dma_start(wat[:, k, :], wat_d[k * 128:(k + 1) * 128, :])
                nc.sync.dma_start(wbt[:, k, :], wbt_d[k * 128:(k + 1) * 128, :])
            nc.sync.dma_start(wrt[:], wrt_d.rearrange("(k p) c -> p k c",
                                                      p=128))
            bab = wp.tile([128, C], F32, tag="bab")
            nc.sync.dma_start(bab[:], bab_d[:])
            br = wp.tile([128, KT], F32, tag="br")
            nc.sync.dma_start(br[:], br_d[:])

            for b in range(BPC):
                if b == 0:
                    xs = xs0
                else:
                    xs = xp.tile([128, KT, N], F32R, tag="xs")
                    for h in range(NS):
                        hsl = slice(h * 512, (h + 1) * 512)
                        nc.sync.dma_start(
                            xs[:, :, hsl],
                            x_d[b, :, hsl].rearrange("(k p) n -> p k n",
                                                     p=128))

                at = ip.tile([128, NT, C], F32R, tag="at")
                ebt = ip.tile([128, NT, C], F32R, tag="ebt")
                ev = ip2.tile([128, KT, N], F32R, tag="ev")
                gt = ip.tile([128, KT, C], F32R, tag="gt")
                zs = ip.tile([128, KT, N], F32R, tag="zs")
                av = sp.tile([128, KT, NS], F32, tag="av")
                svc = sp.tile([128, KT], F32, tag="svc")
                sbc = sp.tile([128, KT], F32, tag="sbc")
                prod = sp.tile([128, KT], F32, tag="prod")
                rsc = sp.tile([128, KT], F32, tag="rsc")
                rsv = sp.tile([128, KT], F32, tag="rsv")
                sbr = sp.tile([1, C], F32R, tag="sbr")
                os_ = op_.tile([128, KT, N], F32, tag="os")

                # Phase V: EV[d,n] natural + per-row expsums (h outer so
                # the first groups only need the first half of x)
                for h in range(NS):
                    hsl = slice(h * 512, (h + 1) * 512)
                    for dt in range(KT):
                        dsl = slice(dt * 128, (dt + 1) * 128)
                        psv = pp.tile([128, 512], F32, tag="mm")
                        for k in range(KT):
                            nc.tensor.matmul(psv[:], wvt[:, k, dsl],
                                             xs[:, k, hsl],
                                             start=(k == 0), stop=(k == KT - 1))
                        nc.scalar.activation(ev[:, dt, hsl], psv[:], AF.Exp,
                                             accum_out=av[:, dt, h:h + 1])
                nc.vector.tensor_add(svc[:], av[:, :, 0], av[:, :, 1])
                nc.vector.reciprocal(rsv[:], svc[:])

                # Phase 1: AT[n,c] and EBT[n,d] per n-tile.
                # k-paired order: consecutive matmuls share the same stationary
                # xs chunk (one weight set serves psa and psb).
                for nt in range(NT):
                    nsl = slice(nt * 128, (nt + 1) * 128)
                    psa = pp.tile([128, C], F32, tag="mm")
                    psb = pp.tile([128, C], F32, tag="mm")
                    for k in range(KT):
                        nc.tensor.matmul(psa[:], xs[:, k, nsl], wat[:, k, :],
                                         start=(k == 0), stop=(k == KT - 1))
                        nc.tensor.matmul(psb[:], xs[:, k, nsl], wbt[:, k, :],
                                         start=(k == 0), stop=(k == KT - 1))
                    nc.vector.tensor_copy(at[:, nt, :], psa[:])
                    nc.scalar.activation(ebt[:, nt, :], psb[:], AF.Exp)

                # Phase 2: sB row via ones-matmul, then to column via K=1 matmuls
                pss = pq.tile([128, 512], F32, tag="q")
                for nt in range(NT):
                    nc.tensor.matmul(pss[:], ones[:], ebt[:, nt, :],
                                     start=(nt == 0), stop=(nt == NT - 1))
                nc.vector.tensor_copy(sbr[:], pss[0:1, :])

                # Phase G: GrawT[d,c]; evac folds the 1/(sB*sV) scale and the
                # +bA[c] bias (GT = GrawT*rscale + bA_bcast*rsV, rank-1-free).
                # The sB row->column transpose matmuls slot in after the first
                # group so their sbr/rsc dependency chain hides under PG.
                for dt in range(KT):
                    dsl = slice(dt * 128, (dt + 1) * 128)
                    psg = pp.tile([128, C], F32, tag="mm")
                    for nt in range(NT):
                        nc.tensor.matmul(psg[:], ebt[:, nt, dsl], at[:, nt, :],
                                         start=(nt == 0), stop=(nt == NT - 1))
                    if dt == 0:
                        psc = pq.tile([128, KT, 2], F32, tag="q")
                        for dtc in range(KT):
                            nc.tensor.matmul(
                                psc[:, dtc, :],
                                sbr[0:1, dtc * 128:(dtc + 1) * 128],
                                ones[0:1, 0:2], start=True, stop=True)
                        nc.vector.tensor_copy(sbc[:], psc[:, :, 0])
                        nc.vector.tensor_mul(prod[:], sbc[:], svc[:])
                        nc.vector.reciprocal(rsc[:], prod[:])
                    gta = sp.tile([128, C], F32, tag="gta")
                    nc.scalar.mul(gta[:], psg[:], rsc[:, dt:dt + 1])
                    tmpb = sp.tile([128, C], F32, tag="tmpb")
                    nc.vector.tensor_scalar_mul(tmpb[:], bab[:],
                                                rsv[:, dt:dt + 1])
                    nc.vector.tensor_add(gt[:, dt, :], gta[:], tmpb[:])

                # Phase Z: Z0[c,n]
                for ct in range(KT):
                    csl = slice(ct * 128, (ct + 1) * 128)
                    for h in range(NS):
                        hsl = slice(h * 512, (h + 1) * 512)
                        psz = pp.tile([128, 512], F32, tag="mm")
                        for dt in range(KT):
                            nc.tensor.matmul(psz[:], gt[:, dt, csl],
                                             ev[:, dt, hsl],
                                             start=(dt == 0), stop=(dt == KT - 1))
                        nc.vector.tensor_copy(zs[:, ct, hsl], psz[:])

                # Phase R: out[o,n] = wR @ Z + bR
                for ot in range(KT):
                    osl = slice(ot * 128, (ot + 1) * 128)
                    for h in range(NS):
                        hsl = slice(h * 512, (h + 1) * 512)
                        psr = pp.tile([128, 512], F32, tag="mm")
                        for k in range(KT):
                            nc.tensor.matmul(psr[:], wrt[:, k, osl],
                                             zs[:, k, hsl],
                                             start=(k == 0), stop=(k == KT - 1))
                        nc.scalar.activation(os_[:, ot, hsl], psr[:],
                                             AF.Identity, bias=br[:, ot:ot + 1])
                        nc.sync.dma_start(
                            o_d[b, ot * 128:(ot + 1) * 128, h * 512:(h + 1) * 512],
                            os_[:, ot, hsl])
    nc.compile()
    return nc


def _in_maps(x, wA, bA, wB, wV, wR, bR):
    xr = np.ascontiguousarray(x.reshape(B, C, N), dtype=np.float32)
    wat = np.ascontiguousarray(wA.T, dtype=np.float32)
    wbt = np.ascontiguousarray(wB.T, dtype=np.float32)
    wvt = np.ascontiguousarray(wV.T, dtype=np.float32)
    wrt = np.ascontiguousarray(wR.T, dtype=np.float32)
    bab = np.ascontiguousarray(
        np.broadcast_to(bA.reshape(1, C), (128, C)), dtype=np.float32)
    br = np.ascontiguousarray(bR.reshape(KT, 128).T, dtype=np.float32)
    ones = np.ones((128, 128), dtype=np.float32)
    maps = []
    for i in range(NCORES):
        maps.append({
            "x": np.ascontiguousarray(xr[i * BPC:(i + 1) * BPC]),
            "wat": wat, "wbt": wbt, "wvt": wvt, "wrt": wrt,
            "bab": bab, "br": br, "ones": ones,
        })
    return maps


def kernel(x, wA, bA, wB, bB, wV, bV, wR, bR):
    from concourse.bass_utils import run_bass_kernel_spmd
    if "nc" not in _CACHE:
        _CACHE["nc"] = _build_nc()
    nc = _CACHE["nc"]
    maps = _in_maps(x, wA, bA, wB, wV, wR, bR)
    res = run_bass_kernel_spmd(nc, maps, list(range(NCORES)))
    out = np.concatenate([res.results[i]["o"] for i in range(NCORES)], axis=0)
    return out.reshape(B, C, H, W).astype(np.float32)
